# revision 14
# baseline (speedup 1.0000x reference)
"""Trainium2 Bass kernel for ragged positional-encoding expansion.

Problem: x (32, 512) int32 per-token durations in [0, 16); pos_enc (5000, 384)
f32 sinusoidal table.  Output (B, T, D): for each batch row, each token i
emits x[b,i] consecutive frames carrying pos_enc[0], pos_enc[1], ...;
T = max total frames over rows; frames past a row's total are zeros.

Key facts exploited:
  * durations <= 15  ->  only pos_enc[0:16] is ever read
  * within-token position pos[t] = t - (largest token start <= t), and that
    start is always within the previous 16 frames -> a scatter of starts +
    a 16-wide windowed max recovers pos everywhere
  * output rows are generated on-chip by a one-hot (16) x table16 matmul,
    so HBM traffic is (nearly) writes only

Sharding: data-parallel over batch. 8 cores x 4 rows each. Full inputs in,
full output out; all device work in a single SPMD Bass/Tile program.
"""

import math
from contextlib import ExitStack

import numpy as np

B, N, D = 32, 512, 384
RPC = 4  # rows per core
NCORES = 8
MAXD = 16  # durations are < MAXD
SEGF = 128  # frames per segment (= matmul M and PE tile height)
SEGS_PER_ROW = 32  # segments processed per pass (RPC * SEGS_PER_ROW = 128)
HALO = 16  # left halo for the windowed max
SCATW = SEGF + HALO  # local_scatter dest width per segment
KILL = -20000.0  # additive kill constant (keeps int16 in range)


def _consts():
    import ml_dtypes

    bf16 = ml_dtypes.bfloat16
    q = np.arange(128)
    # selg[q, 128*g + m] = 1 iff (q % 32) == 4g + m//32 ; used as matmul lhsT
    # slice (32,128)@base 32r to replicate group g's 4 segments x32 each.
    sel = np.zeros((128, 8 * 128), dtype=bf16)
    for g in range(8):
        for m in range(128):
            k = 4 * g + m // 32
            sel[(q % 32) == k, 128 * g + m] = 1
    segb = ((q % SEGS_PER_ROW) * SEGF).astype(np.float32)[:, None]  # (128,1)
    iota16 = (q % MAXD).astype(np.float32)[:, None]  # (128,1)
    iotaF = np.tile(np.arange(SEGF, dtype=np.float32), (128, 1))  # (128,128)
    return sel, segb, iota16, iotaF


def _table4(pos_enc):
    """pos_enc[0:16] split hi/lo in bf16, stacked to (32,D), tiled x4 -> (128,D)."""
    import ml_dtypes

    bf16 = ml_dtypes.bfloat16
    t = np.asarray(pos_enc[:MAXD], dtype=np.float32)  # (16, D)
    hi = t.astype(bf16)
    lo = (t - hi.astype(np.float32)).astype(bf16)
    tab = np.concatenate([hi, lo], axis=0)  # (32, D) bf16
    return np.tile(tab, (4, 1))  # (128, D) bf16


def build(T):
    """Build + compile the per-core SPMD program for a given T."""
    import concourse.mybir as mybir
    import concourse.tile as tile
    from concourse import bacc

    dt = mybir.dt
    Alu = mybir.AluOpType

    TP = ((T + SEGF - 1) // SEGF) * SEGF
    S = TP // SEGF  # total segments per row
    NPASS = (S + SEGS_PER_ROW - 1) // SEGS_PER_ROW
    PASSF = SEGS_PER_ROW * SEGF  # frames per pass (4096)

    nc = bacc.Bacc(
        "TRN2",
        target_bir_lowering=False,
        debug=False,
        num_devices=NCORES,
    )

    x_d = nc.dram_tensor("x", [RPC, N], dt.int32, kind="ExternalInput").ap()
    sel_d = nc.dram_tensor("c_sel", [128, 8 * 128], dt.bfloat16, kind="ExternalInput").ap()
    segb_d = nc.dram_tensor("c_segb", [128, 1], dt.float32, kind="ExternalInput").ap()
    i16_d = nc.dram_tensor("c_i16", [128, 1], dt.float32, kind="ExternalInput").ap()
    iof_d = nc.dram_tensor("c_iof", [128, SEGF], dt.float32, kind="ExternalInput").ap()
    tab_d = nc.dram_tensor("c_tab", [128, D], dt.bfloat16, kind="ExternalInput").ap()
    out_d = nc.dram_tensor("out", [RPC, TP, D], dt.float32, kind="ExternalOutput").ap()
    # DRAM bounce buffers for partition replication (row r -> partitions 32r..32r+31)
    skb_d = nc.dram_tensor("skb", [RPC, N], dt.float32).ap()
    stb_d = nc.dram_tensor("stb", [RPC, N], dt.float32).ap()
    totb_d = nc.dram_tensor("totb", [RPC, 1], dt.float32).ap()

    with ExitStack() as ctx:
        tc = ctx.enter_context(tile.TileContext(nc))
        cp = ctx.enter_context(tc.tile_pool(name="consts", bufs=1))
        wp = ctx.enter_context(tc.tile_pool(name="work", bufs=1))
        pp = ctx.enter_context(tc.tile_pool(name="perpass", bufs=min(2, NPASS)))
        pbp = ctx.enter_context(tc.tile_pool(name="pb", bufs=2, space="PSUM"))
        omp = ctx.enter_context(tc.tile_pool(name="om", bufs=4, space="PSUM"))
        ohp = ctx.enter_context(tc.tile_pool(name="oh", bufs=3))
        obp = ctx.enter_context(tc.tile_pool(name="ob", bufs=3))

        # ---- constants to SBUF
        sel = cp.tile([128, 8 * 128], dt.bfloat16)
        nc.sync.dma_start(out=sel, in_=sel_d)
        segb = cp.tile([128, 1], dt.float32)
        nc.sync.dma_start(out=segb, in_=segb_d)
        i16 = cp.tile([128, 1], dt.float32)
        nc.sync.dma_start(out=i16, in_=i16_d)
        iof = cp.tile([128, SEGF], dt.float32)
        nc.sync.dma_start(out=iof, in_=iof_d)
        tab = cp.tile([128, D], dt.bfloat16)
        nc.sync.dma_start(out=tab, in_=tab_d)

        # ---- prologue: cumsum, starts, totals, dup-kill, replication
        xi = wp.tile([RPC, N], dt.int32)
        nc.sync.dma_start(out=xi, in_=x_d)

        cur = xi
        flip = True
        for k in (1, 2, 4, 8, 16, 32, 64, 128, 256):
            nxt = wp.tile([RPC, N], dt.int32, tag="cs_a" if flip else "cs_b", name=f"cs{k}")
            flip = not flip
            nc.vector.tensor_copy(out=nxt[:, :k], in_=cur[:, :k])
            nc.vector.tensor_tensor(
                out=nxt[:, k:], in0=cur[:, k:], in1=cur[:, : N - k], op=Alu.add
            )
            cur = nxt
        cs = cur  # (RPC, N) int32 inclusive cumsum

        starts_f = wp.tile([RPC, N], dt.float32)
        st_i = wp.tile([RPC, N], dt.int32)
        nc.vector.tensor_tensor(out=st_i, in0=cs, in1=xi, op=Alu.subtract)
        nc.vector.tensor_copy(out=starts_f, in_=st_i)  # cast to f32

        totals_f = wp.tile([RPC, 1], dt.float32)
        nc.vector.tensor_copy(out=totals_f, in_=cs[:, N - 1 : N])

        # duplicate-start kill: token i (i>=1) is a duplicate iff x[i-1]==0
        xf = wp.tile([RPC, N], dt.float32)
        nc.vector.tensor_copy(out=xf, in_=xi)
        z = wp.tile([RPC, N - 1], dt.float32)
        nc.vector.tensor_scalar(
            out=z, in0=xf[:, : N - 1], scalar1=0.0, scalar2=None, op0=Alu.is_equal
        )
        zk = wp.tile([RPC, N - 1], dt.float32)
        nc.vector.tensor_scalar(
            out=zk, in0=z, scalar1=KILL, scalar2=None, op0=Alu.mult
        )
        sk = wp.tile([RPC, N], dt.float32)
        nc.vector.tensor_copy(out=sk[:, :1], in_=starts_f[:, :1])
        nc.vector.tensor_tensor(out=sk[:, 1:], in0=starts_f[:, 1:], in1=zk, op=Alu.add)

        # replicate each row to its 32 segment partitions via a DRAM bounce
        # (DMA reads the row 32x with a zero-step dim; SBUF-side zero-step
        # partition APs are not allowed, DRAM-side ones are fine)
        from concourse.bass import AP as BassAP

        nc.sync.dma_start(out=skb_d, in_=sk)
        nc.sync.dma_start(out=stb_d, in_=starts_f)
        nc.sync.dma_start(out=totb_d, in_=totals_f)
        repl = wp.tile([128, N], dt.float32)
        vrepl = wp.tile([128, N], dt.float32)
        totrep = wp.tile([128, 1], dt.float32)
        nc.sync.dma_start(
            out=repl, in_=BassAP(skb_d.tensor, 0, [[N, RPC], [0, SEGS_PER_ROW], [1, N]])
        )
        nc.sync.dma_start(
            out=vrepl, in_=BassAP(stb_d.tensor, 0, [[N, RPC], [0, SEGS_PER_ROW], [1, N]])
        )
        nc.sync.dma_start(
            out=totrep, in_=BassAP(totb_d.tensor, 0, [[1, RPC], [0, SEGS_PER_ROW], [1, 1]])
        )

        val16 = wp.tile([128, N], dt.int16)
        nc.vector.tensor_copy(out=val16, in_=vrepl)

        # ---- per pass: scatter -> window max -> pos -> one-hot -> matmul -> DMA
        for p in range(NPASS):
            pbase = float(p * PASSF)

            idx_f = pp.tile([128, N], dt.float32)
            nc.vector.tensor_scalar(
                out=idx_f, in0=repl, scalar1=segb, scalar2=float(HALO) - pbase,
                op0=Alu.subtract, op1=Alu.add,
            )
            geh = pp.tile([128, N], dt.float32)
            nc.vector.tensor_scalar(
                out=geh, in0=idx_f, scalar1=float(SCATW), scalar2=KILL,
                op0=Alu.is_ge, op1=Alu.mult,
            )
            idx2 = pp.tile([128, N], dt.float32)
            nc.vector.tensor_tensor(out=idx2, in0=idx_f, in1=geh, op=Alu.add)
            idx16 = pp.tile([128, N], dt.int16)
            nc.vector.tensor_copy(out=idx16, in_=idx2)

            scat = pp.tile([128, SCATW], dt.int16)
            nc.gpsimd.local_scatter(
                scat, val16, idx16, channels=128, num_elems=SCATW, num_idxs=N
            )

            b1 = pp.tile([128, SCATW - 1], dt.int16)
            nc.vector.tensor_tensor(
                out=b1, in0=scat[:, 1:SCATW], in1=scat[:, : SCATW - 1], op=Alu.max
            )
            b2 = pp.tile([128, SCATW - 3], dt.int16)
            nc.vector.tensor_tensor(
                out=b2, in0=b1[:, 2:], in1=b1[:, : SCATW - 3], op=Alu.max
            )
            b4 = pp.tile([128, SCATW - 7], dt.int16)
            nc.vector.tensor_tensor(
                out=b4, in0=b2[:, 4:], in1=b2[:, : SCATW - 7], op=Alu.max
            )
            b8 = pp.tile([128, SCATW - 15], dt.int16)
            nc.vector.tensor_tensor(
                out=b8, in0=b4[:, 8:], in1=b4[:, : SCATW - 15], op=Alu.max
            )

            wm_f = pp.tile([128, SEGF], dt.float32)
            nc.vector.tensor_copy(out=wm_f, in_=b8[:, 1 : 1 + SEGF])

            e = pp.tile([128, SEGF], dt.float32)
            nc.vector.tensor_scalar(
                out=e, in0=wm_f, scalar1=segb, scalar2=pbase,
                op0=Alu.subtract, op1=Alu.subtract,
            )
            pos = pp.tile([128, SEGF], dt.float32)
            nc.vector.tensor_tensor(out=pos, in0=iof, in1=e, op=Alu.subtract)

            thr = pp.tile([128, 1], dt.float32)
            nc.vector.tensor_scalar(
                out=thr, in0=totrep, scalar1=segb, scalar2=pbase,
                op0=Alu.subtract, op1=Alu.subtract,
            )
            gev = pp.tile([128, SEGF], dt.float32)
            nc.vector.tensor_scalar(
                out=gev, in0=iof, scalar1=thr, scalar2=16.0,
                op0=Alu.is_ge, op1=Alu.mult,
            )
            posq = pp.tile([128, SEGF], dt.float32)
            nc.vector.tensor_tensor(out=posq, in0=pos, in1=gev, op=Alu.add)
            posm = pp.tile([128, SEGF], dt.bfloat16)
            nc.vector.tensor_scalar(
                out=posm, in0=posq, scalar1=float(MAXD), scalar2=None, op0=Alu.min
            )

            # tiles actually needed this pass (tiles fully past T are skipped)
            frames_left = T - p * PASSF
            n_tiles = max(0, min(SEGS_PER_ROW, (frames_left + SEGF - 1) // SEGF))
            n_groups = (n_tiles + 3) // 4

            for r in range(RPC):
                rbase = SEGS_PER_ROW * r
                for g in range(n_groups):
                    nt = min(4, n_tiles - 4 * g)
                    pb = pbp.tile([128, SEGF], dt.float32)
                    nc.tensor.matmul(
                        out=pb,
                        lhsT=sel[rbase : rbase + 32, 128 * g : 128 * (g + 1)],
                        rhs=posm[rbase : rbase + 32, :],
                        start=True,
                        stop=True,
                        tile_position=(rbase, 0),
                    )
                    oh = ohp.tile([128, SEGF], dt.bfloat16)
                    nc.vector.tensor_scalar(
                        out=oh, in0=pb, scalar1=i16, scalar2=None, op0=Alu.is_equal
                    )
                    ob = obp.tile([128, 4 * D], dt.float32)
                    for k in range(nt):
                        om = omp.tile([128, D], dt.float32)
                        nc.tensor.matmul(
                            out=om,
                            lhsT=oh[32 * k : 32 * k + 32, :],
                            rhs=tab[32 * k : 32 * k + 32, :],
                            start=True,
                            stop=True,
                            tile_position=(32 * k, 0),
                        )
                        dst = ob[:, D * k : D * (k + 1)]
                        if (g + k) % 2 == 0:
                            nc.vector.tensor_copy(out=dst, in_=om)
                        else:
                            nc.scalar.copy(out=dst, in_=om)
                    f0 = p * PASSF + g * 512
                    dram = out_d[r, f0 : f0 + nt * SEGF, :].rearrange(
                        "(a q) d -> q a d", q=SEGF
                    )
                    sbv = ob[:, : nt * D].rearrange("q (a d) -> q a d", d=D)
                    nc.sync.dma_start(out=dram, in_=sbv)

    nc.compile()
    return nc, TP


def _in_maps(x_np, pos_enc_np):
    sel, segb, i16, iof = _consts()
    tab = _table4(pos_enc_np)
    maps = []
    for c in range(NCORES):
        maps.append(
            {
                "x": np.ascontiguousarray(x_np[RPC * c : RPC * (c + 1)]),
                "c_sel": sel,
                "c_segb": segb,
                "c_i16": i16,
                "c_iof": iof,
                "c_tab": tab,
            }
        )
    return maps


def run_hw(x, pos_enc, trace=False):
    """Run on the 8 NeuronCores; returns (full_output, BassKernelResults)."""
    from concourse.bass_utils import run_bass_kernel_spmd

    x_np = np.asarray(x, dtype=np.int32)
    pe = np.asarray(pos_enc, dtype=np.float32)
    T = int(np.cumsum(x_np.astype(np.int64), axis=1).max())
    nc, TP = build(T)
    res = run_bass_kernel_spmd(nc, _in_maps(x_np, pe), list(range(NCORES)), trace=trace)
    outs = [np.asarray(res.results[c]["out"])[:, :T, :] for c in range(NCORES)]
    return np.concatenate(outs, axis=0).astype(np.float32, copy=False), res


def kernel(**inputs):
    out, _ = run_hw(inputs["x"], inputs["pos_enc"], trace=False)
    return out


def run_hw_timed(x, pos_enc, iters=10):
    """Run once for output, then time `iters` chained executions on device.

    Returns (full_output, per_iteration_seconds). Inputs are device-resident
    and outputs are not fetched during the timed loop, so per-iteration time
    ~= device execution + amortized dispatch.
    """
    import time as _time

    import jax
    from jax.experimental.shard_map import shard_map
    from jax.sharding import Mesh, NamedSharding, PartitionSpec

    import concourse.mybir as mybir
    from concourse.bass2jax import (
        _bass_exec_p,
        install_neuronx_cc_hook,
        partition_id_tensor,
    )

    x_np = np.asarray(x, dtype=np.int32)
    pe = np.asarray(pos_enc, dtype=np.float32)
    T = int(np.cumsum(x_np.astype(np.int64), axis=1).max())
    nc, TP = build(T)
    in_maps = _in_maps(x_np, pe)

    install_neuronx_cc_hook()
    assert nc.dbg_addr is None
    partition_name = nc.partition_id_tensor.name if nc.partition_id_tensor else None

    in_names, out_names, out_avals, zero_outs = [], [], [], []
    for alloc in nc.m.functions[0].allocations:
        if not isinstance(alloc, mybir.MemoryLocationSet):
            continue
        name = alloc.memorylocations[0].name
        if alloc.kind == "ExternalInput":
            if name != partition_name:
                in_names.append(name)
        elif alloc.kind == "ExternalOutput":
            out_names.append(name)
            shape = tuple(alloc.tensor_shape)
            dtype = mybir.dt.np(alloc.dtype)
            out_avals.append(jax.core.ShapedArray(shape, dtype))
            zero_outs.append(np.zeros(shape, dtype))
    n_params = len(in_names)
    all_names = in_names + out_names
    if partition_name is not None:
        all_names = all_names + [partition_name]

    def _body(*args):
        operands = list(args)
        if partition_name is not None:
            operands.append(partition_id_tensor())
        outs = _bass_exec_p.bind(
            *operands,
            out_avals=tuple(out_avals),
            in_names=tuple(all_names),
            out_names=tuple(out_names),
            lowering_input_output_aliases=(),
            sim_require_finite=True,
            sim_require_nnan=True,
            nc=nc,
        )
        return tuple(outs)

    devices = jax.devices()[:NCORES]
    mesh = Mesh(np.asarray(devices), ("core",))
    nspec = (PartitionSpec("core"),)
    fn = jax.jit(
        shard_map(
            _body,
            mesh=mesh,
            in_specs=nspec * (n_params + len(out_names)),
            out_specs=nspec * len(out_names),
            check_rep=False,
        ),
        keep_unused=True,
    )
    sh = NamedSharding(mesh, PartitionSpec("core"))
    concat_in = [
        np.concatenate([np.asarray(in_maps[c][nm]) for c in range(NCORES)], axis=0)
        for nm in in_names
    ]
    concat_zero = [np.zeros((NCORES * z.shape[0], *z.shape[1:]), z.dtype) for z in zero_outs]
    dev_args = [jax.device_put(a, sh) for a in concat_in + concat_zero]

    outs = fn(*dev_args)
    jax.block_until_ready(outs)
    full = np.asarray(outs[out_names.index("out")]).reshape(NCORES * RPC, TP, D)[:, :T, :]

    t0 = _time.perf_counter()
    last = None
    for _ in range(iters):
        last = fn(*dev_args)
    jax.block_until_ready(last)
    per_iter = (_time.perf_counter() - t0) / iters
    return np.ascontiguousarray(full), per_iter
